# revision 35
# baseline (speedup 1.0000x reference)
"""Trainium2 Bass kernel for nn_Attention_44504451121208.

Dual-stream (x / x_hsi) 12-head attention block:
  qkv -> [template-template attn (shared), search-all attn per stream] -> proj.

Strategy: data-parallel over batch B=64 across 8 NeuronCores (8 batches/core,
no collectives). All matmuls in bf16 (fp32 accumulation in PSUM).

Layout notes (per core):
 - Tokens are reordered internally to [search 256 | template 128] so the
   template-key chunk is a single contraction tile and the template-query
   columns ride along the search columns in one matmul.
 - x is cast to bf16 straight in DRAM via SWDGE cast-DMA (two block-copies
   that also apply the token reorder), then read back with XBAR DMA-transpose
   to give x^T [C, N] tiles (contraction over C needs C on partitions).
 - q^T,k^T computed head-major [C, N] (weights stationary); v computed
   token-major [N, C] (x^T stationary) with a ones-column appended per head
   so the attention-value matmul also emits softmax denominators (row 64).
 - Scores are computed key-major S^T = k @ q^T with the even/odd head of each
   pair issued back-to-back at PE row-groups 0-63 / 64-127 so the two 64-row
   matmuls run concurrently in the systolic array; exp on ScalarE (logits are
   tiny: |s|<~3, so no max subtraction, matching softmax semantics exactly
   up to fp rounding); O^T = (v_aug)^T @ exp(S^T) accumulates over key chunks.
 - Normalization is batched per batch-iteration: each head's denominator row
   is copied (ScalarE) into a [12, 640] collector, one VectorE reciprocal
   serves all 24 attend calls, a constant 0/1 selector matmul broadcasts the
   reciprocals across the 128 partitions of each channel chunk, and one
   multiply per chunk half rescales the raw O^T tiles. No GpSimd in the loop
   and no single-partition reciprocals.
 - The normalize+proj of batch b-1 is interleaved into batch b's attend phase
   so the PE never waits on the normalization chain.
 - proj consumes O^T head-major directly (stationary), emits token-major
   tiles, adds bias broadcast, DMAs straight to the outputs. The template
   block is computed once and written to both outputs.
"""

import sys

sys.path.insert(0, "/opt/trn_rl_repo")

import numpy as np

B, N, C = 64, 384, 768
H, DH = 12, 64
LT, LS = 128, 256  # template / search token counts
NCORES = 8
BL = B // NCORES  # batches per core
CK = C // 128  # contraction chunks
NT = N // 128  # token tiles
C3 = 3 * C

_CACHE = {}


def _build_program(variant="default"):
    import concourse.tile as tile
    from concourse import bacc, library_config
    import concourse.mybir as mybir

    dt = mybir.dt
    BF, F32 = dt.bfloat16, dt.float32
    Exp = mybir.ActivationFunctionType.Exp

    nc = bacc.Bacc("TRN2", target_bir_lowering=False, debug=False)

    x_in = nc.dram_tensor("x", [BL, N, C], F32, kind="ExternalInput")
    xh_in = nc.dram_tensor("x_hsi", [BL, N, C], F32, kind="ExternalInput")
    qkvw_in = nc.dram_tensor("qkv_w", [C3, C], F32, kind="ExternalInput")
    projw_in = nc.dram_tensor("proj_w", [C, C], F32, kind="ExternalInput")
    projb_in = nc.dram_tensor("proj_b", [C], F32, kind="ExternalInput")
    out0 = nc.dram_tensor("out", [BL, N, C], F32, kind="ExternalOutput")
    out1 = nc.dram_tensor("out_hsi", [BL, N, C], F32, kind="ExternalOutput")

    with tile.TileContext(nc) as tc:
        with tc.tile_critical():
            nc.gpsimd.load_library(library_config.attn)

        import contextlib

        stack = contextlib.ExitStack()
        with stack:
            const = stack.enter_context(tc.tile_pool(name="const", bufs=1))
            dram = stack.enter_context(tc.tile_pool(name="dram", bufs=6, space="DRAM"))
            wdram = stack.enter_context(tc.tile_pool(name="wdram", bufs=1, space="DRAM"))
            xtp = stack.enter_context(tc.tile_pool(name="xtp", bufs=15))
            qkp = stack.enter_context(tc.tile_pool(name="qkp", bufs=36))
            vbp = stack.enter_context(tc.tile_pool(name="vbp", bufs=9))
            atp = stack.enter_context(tc.tile_pool(name="atp", bufs=4))
            denp = stack.enter_context(tc.tile_pool(name="denp", bufs=2))
            recp = stack.enter_context(tc.tile_pool(name="recp", bufs=4))
            obp = stack.enter_context(tc.tile_pool(name="obp", bufs=12))
            obp2 = stack.enter_context(tc.tile_pool(name="obp2", bufs=12))
            osbp = stack.enter_context(tc.tile_pool(name="osbp", bufs=4))
            psmm = stack.enter_context(tc.tile_pool(name="psmm", bufs=3, space="PSUM"))
            pss = stack.enter_context(tc.tile_pool(name="pss", bufs=3, space="PSUM"))
            pso = stack.enter_context(tc.tile_pool(name="pso", bufs=2, space="PSUM"))

            def stage_matrix(src, b):
                """cast one [N, C] fp32 matrix to bf16 in DRAM (SWDGE), tokens
                reordered to [search | template]; return x^T SBUF tiles."""
                xb = dram.tile([N, C], BF, tag="xb")
                nc.gpsimd.dma_start(xb[0:256, :], src[b, 128:384, :])
                nc.gpsimd.dma_start(xb[256:384, :], src[b, 0:128, :])
                xt = []
                for ci in range(CK):
                    t = xtp.tile([128, N], BF, tag="xt")
                    nc.sync.dma_start_transpose(t[:], xb[:, ci * 128 : (ci + 1) * 128])
                    xt.append(t)
                return xt

            # stage batch 0's activations first so their casts/transposes
            # aren't stuck behind the weight pipeline in the DMA queues
            xt0_m = stage_matrix(x_in, 0)
            xt0_h = stage_matrix(xh_in, 0)

            # ---------------- weights ----------------
            # cast qkv_w / proj_w to bf16 DRAM copies via SWDGE cast-DMA, then
            # XBAR-transpose them back so the contraction dim (input C) is on
            # partitions.
            qkv_wb = wdram.tile([C3, C], BF, tag="qkv_wb")
            proj_wb = wdram.tile([C, C], BF, tag="proj_wb")
            nc.gpsimd.dma_start(qkv_wb[:], qkvw_in[:])
            nc.gpsimd.dma_start(proj_wb[:], projw_in[:])

            wt = []  # qkv_w^T chunks: wt[ci] = [128 (C rows ci), 2304]
            wpt = []  # proj_w^T chunks: wpt[ci] = [128, 768]
            for ci in range(CK):
                t = const.tile([128, C3], BF, tag=f"wt{ci}")
                nc.sync.dma_start_transpose(t[:], qkv_wb[:, ci * 128 : (ci + 1) * 128])
                wt.append(t)
            for ci in range(CK):
                t = const.tile([128, C], BF, tag=f"wpt{ci}")
                nc.sync.dma_start_transpose(t[:], proj_wb[:, ci * 128 : (ci + 1) * 128])
                wpt.append(t)

            bias1 = const.tile([1, C], F32, tag="bias1")
            nc.sync.dma_start(bias1[:], projb_in[:].unsqueeze(0))
            bias_bc = const.tile([128, C], F32, tag="bias_bc")
            nc.gpsimd.partition_broadcast(bias_bc[:], bias1[:])

            # head-selector constant: E[h, c] = 1 iff c in [64h, 64h+64)
            # (used to broadcast per-head reciprocals onto 128 partitions).
            # Engine writes must start at partition 0/32/64/96, so the
            # per-head rows are placed by DMA via a DRAM ones staging buffer.
            esel = const.tile([12, C], BF, tag="esel")
            nc.vector.memset(esel[:], 0.0)
            ones1 = const.tile([1, 64], BF, tag="ones1")
            nc.vector.memset(ones1[:], 1.0)
            ones_dram = wdram.tile([64], BF, tag="ones_dram")
            nc.sync.dma_start(ones_dram[:].unsqueeze(0), ones1[:])
            for h in range(H):
                nc.sync.dma_start(
                    esel[h : h + 1, h * 64 : (h + 1) * 64],
                    ones_dram[:].unsqueeze(0),
                )

            # ---------------- per-batch pipeline ----------------
            def qkv_matrix(xt, hsi):
                """q^T,k^T head-major tiles + v token-major (65-strided heads
                with a trailing ones column per head)."""
                qk = []
                for m in range(12):
                    fm = 256 if (hsi and m < 6) else 384  # hsi q: search only
                    ps = psmm.tile([128, 384], F32, tag="mm")
                    for ci in range(CK):
                        nc.tensor.matmul(
                            ps[:, :fm],
                            wt[ci][:, m * 128 : (m + 1) * 128],
                            xt[ci][:, :fm],
                            start=(ci == 0),
                            stop=(ci == CK - 1),
                        )
                    t = qkp.tile([128, 384], BF, tag="qk")
                    nc.vector.tensor_copy(t[:, :fm], ps[:, :fm])
                    qk.append(t)
                vb = []
                for it in range(NT):
                    t = vbp.tile([128, 12 * 65], BF, tag="vb")
                    v3 = t[:].rearrange("p (h e) -> p h e", e=65)
                    nc.vector.memset(v3[:, :, 64:65], 1.0)
                    psa = psmm.tile([128, 384], F32, tag="mm")
                    psb = psmm.tile([128, 384], F32, tag="mm")
                    for ci in range(CK):
                        lhs = xt[ci][:, it * 128 : (it + 1) * 128]
                        nc.tensor.matmul(
                            psa[:], lhs, wt[ci][:, 1536:1920],
                            start=(ci == 0), stop=(ci == CK - 1),
                        )
                        nc.tensor.matmul(
                            psb[:], lhs, wt[ci][:, 1920:2304],
                            start=(ci == 0), stop=(ci == CK - 1),
                        )
                    nc.vector.tensor_copy(
                        v3[:, 0:6, 0:64], psa[:].rearrange("p (h e) -> p h e", e=64)
                    )
                    nc.vector.tensor_copy(
                        v3[:, 6:12, 0:64], psb[:].rearrange("p (h e) -> p h e", e=64)
                    )
                    vb.append(t)
                return qk, vb

            def attend_pair(qk, vb, i, hsi, oraw, dstage):
                """heads 2i / 2i+1 of one stream: paired S^T matmuls (row
                groups 0-63 / 64-127 run concurrently) -> exp -> O^T(+denom);
                raw O^T and denominators are stashed for batched
                normalization in finish_norm."""
                nq = 256 if hsi else 384
                co = 384 if hsi else 0
                ats = []
                for po in (0, 64):
                    ats.append(
                        atp.tile(
                            [128, 3 * 384],
                            BF,
                            tag="ath" if hsi else "at",
                            name=f"at_{hsi}_{i}_{po}",
                        )
                    )
                for ck in (2, 0, 1):
                    fq = nq if ck == 2 else 256
                    for j, po in enumerate((0, 64)):
                        sp = pss.tile([128, 384], F32, tag="s")
                        nc.tensor.matmul(
                            sp[:, :fq],
                            qk[6 + i][po : po + 64, ck * 128 : (ck + 1) * 128],
                            qk[i][po : po + 64, :fq],
                            start=True,
                            stop=True,
                        )
                        nc.scalar.activation(
                            ats[j][:, ck * nq : ck * nq + fq],
                            sp[:, :fq],
                            Exp,
                            scale=0.125,
                        )
                for j, po in enumerate((0, 64)):
                    h = 2 * i + j
                    op = pso.tile([65, 384], F32, tag="o")
                    for idx, ck in enumerate((2, 0, 1)):
                        fq = nq if ck == 2 else 256
                        nc.tensor.matmul(
                            op[:, :fq],
                            vb[ck][:, h * 65 : (h + 1) * 65],
                            ats[j][:, ck * nq : ck * nq + fq],
                            start=(idx == 0),
                            stop=(idx == 2),
                            skip_group_check=True,
                        )
                    nc.scalar.copy(
                        dstage[0:1, h * 640 + co : h * 640 + co + nq],
                        op[64:65, :nq],
                    )
                    nc.vector.tensor_copy(
                        oraw[i][po : po + 64, co : co + nq], op[0:64, :nq]
                    )

            def finish_norm(oraw, den_dram):
                """batched softmax normalization for one batch: one wide
                reciprocal, selector-matmul partition broadcast, one multiply
                per channel-chunk half. The denominators arrive via a DRAM
                round-trip that spreads the partition-0 staging row onto 12
                partitions (engine writes can't target arbitrary ones)."""
                den = recp.tile([12, 640], BF, tag="den")
                nc.sync.dma_start(den[:], den_dram[:])
                rec = recp.tile([12, 640], F32, tag="rec")
                nc.vector.reciprocal(rec[:], den[:])
                recb = recp.tile([12, 640], BF, tag="recb")
                nc.vector.tensor_copy(recb[:], rec[:])
                obn = []
                for ci in range(CK):
                    sa = psmm.tile([128, 384], F32, tag="mm")
                    nc.tensor.matmul(
                        sa[:],
                        esel[:, ci * 128 : (ci + 1) * 128],
                        recb[:, 0:384],
                        start=True,
                        stop=True,
                    )
                    sb = psmm.tile([128, 384], F32, tag="mm")
                    nc.tensor.matmul(
                        sb[:, :256],
                        esel[:, ci * 128 : (ci + 1) * 128],
                        recb[:, 384:640],
                        start=True,
                        stop=True,
                    )
                    ob = obp2.tile([128, 640], BF, tag="obn")
                    nc.vector.tensor_mul(ob[:, 0:384], oraw[ci][:, 0:384], sa[:])
                    nc.vector.tensor_mul(
                        ob[:, 384:640], oraw[ci][:, 384:640], sb[:, :256]
                    )
                    obn.append(ob)
                return obn

            # output column ranges of the 5 proj tiles (internal order):
            # 0: main search 0:128   -> out[b, 128:256]
            # 1: main search 128:256 -> out[b, 256:384]
            # 2: template (shared)   -> out[b, 0:128] and out_hsi[b, 0:128]
            # 3: hsi search 0:128    -> out_hsi[b, 128:256]
            # 4: hsi search 128:256  -> out_hsi[b, 256:384]
            def proj(obuf, b):
                targets = [
                    [(out0, 128)],
                    [(out0, 256)],
                    [(out0, 0), (out1, 0)],
                    [(out1, 128)],
                    [(out1, 256)],
                ]
                for tt in range(5):
                    psa = psmm.tile([128, 384], F32, tag="mm")
                    psb = psmm.tile([128, 384], F32, tag="mm")
                    for ci in range(CK):
                        lhs = obuf[ci][:, tt * 128 : (tt + 1) * 128]
                        nc.tensor.matmul(
                            psa[:], lhs, wpt[ci][:, 0:384],
                            start=(ci == 0), stop=(ci == CK - 1),
                        )
                        nc.tensor.matmul(
                            psb[:], lhs, wpt[ci][:, 384:768],
                            start=(ci == 0), stop=(ci == CK - 1),
                        )
                    ob = osbp.tile([128, C], F32, tag="outsb")
                    nc.vector.tensor_add(ob[:, 0:384], psa[:], bias_bc[:, 0:384])
                    nc.vector.tensor_add(ob[:, 384:768], psb[:], bias_bc[:, 384:768])
                    for dst, row in targets[tt]:
                        nc.gpsimd.dma_start(dst[b, row : row + 128, :], ob[:])

            prev = None
            for b in range(BL):
                if b == 0:
                    xt_m, xt_h = xt0_m, xt0_h
                else:
                    xt_m = stage_matrix(x_in, b)
                    xt_h = stage_matrix(xh_in, b)
                qk_m, vb_m = qkv_matrix(xt_m, hsi=False)
                qk_h, vb_h = qkv_matrix(xt_h, hsi=True)
                obn_prev = finish_norm(*prev[:2]) if prev is not None else None
                oraw = [
                    obp.tile([128, 640], BF, tag="oraw", name=f"oraw_{b}_{j}")
                    for j in range(CK)
                ]
                dstage = denp.tile([1, 12 * 640], BF, tag="dstage")
                for i in range(6):
                    attend_pair(qk_m, vb_m, i, False, oraw, dstage)
                    attend_pair(qk_h, vb_h, i, True, oraw, dstage)
                den_dram = dram.tile([12, 640], BF, tag="den_dram")
                nc.sync.dma_start(
                    den_dram[:].rearrange("h q -> (h q)").unsqueeze(0), dstage[:]
                )
                if obn_prev is not None:
                    proj(obn_prev, prev[2])
                prev = (oraw, den_dram, b)
            obn = finish_norm(*prev[:2])
            proj(obn, prev[2])

    nc.compile()
    return nc


def _get_program(variant="default"):
    if variant not in _CACHE:
        _CACHE[variant] = _build_program(variant)
    return _CACHE[variant]


def kernel(x, x_hsi, qkv_w, proj_w, proj_b, t_h=8, t_w=8, s_h=16, s_w=16,
           num_heads=12, **_ignored):
    from concourse.bass_utils import run_bass_kernel_spmd

    nc = _get_program()
    x = np.asarray(x, dtype=np.float32)
    x_hsi = np.asarray(x_hsi, dtype=np.float32)
    qkv_w = np.asarray(qkv_w, dtype=np.float32)
    proj_w = np.asarray(proj_w, dtype=np.float32)
    proj_b = np.asarray(proj_b, dtype=np.float32)

    core_ids = list(range(NCORES))
    in_maps = [
        {
            "x": x[c * BL : (c + 1) * BL],
            "x_hsi": x_hsi[c * BL : (c + 1) * BL],
            "qkv_w": qkv_w,
            "proj_w": proj_w,
            "proj_b": proj_b,
        }
        for c in core_ids
    ]
    res = run_bass_kernel_spmd(nc, in_maps, core_ids)
    out = np.concatenate([res.results[c]["out"] for c in core_ids], axis=0)
    out_hsi = np.concatenate([res.results[c]["out_hsi"] for c in core_ids], axis=0)
    return out, out_hsi
